# revision 9
# baseline (speedup 1.0000x reference)
"""Trainium2 Bass kernel for LowDimProjectedAttention (v2).

Model (reference):
  Q = x @ Wq.T ; K,V likewise  (d_model=2048 -> r=512)
  16 heads of d_k=32, softmax(QK^T/sqrt(32)) @ V, then out_proj r->d_model.
  B=2, S=2048. mask is all-ones (spec fill), dropout p=0, biases zero but
  still applied.

Sharding (8 cores): core c handles batch b=c//4 and the 4 heads j=c%4
(128 of the 512 r-channels, column-parallel QKV). Attention is fully
local. A 4-way AllGather per q tile rebuilds attn_out (bf16), then each
core computes a 512-wide slice of d_model (column-parallel out_proj).

v2 structure (vs v1):
  - x and all weights are bf16 (host-cast): halves DMA and SBUF.
  - K,V projections run weight-stationary (dm-outer, 4 token tiles inner)
    so consecutive matmuls reuse the stationary operand; Q likewise.
  - Softmax denominator is folded into the AV matmul: the stationary is
    [V_chunk | ones] (64 cols), so out rows 0-31 are AV and 32-63 the
    denominator broadcast - the separate ones-matmul stream is gone.
  - exp is split between ACT (exact, bf16 out) and DVE (Schraudolph
    int16 bit-trick approximating exp in bf16), removing the Scalar
    engine as the per-tile bottleneck.
  - out_proj for tile q is emitted after attention of tile q+1 so the
    AllGather never head-of-line blocks the PE queue.
"""

import math

import numpy as np

B = 2
S = 2048
D_MODEL = 2048
R = 512
N_HEADS = 16
D_K = 32
N_CORES = 8
GROUP = 4          # cores per batch group
RLOC = 128         # r-channels per core (4 heads x 32)
NH = 4             # heads per core
TQ = 512           # q tile size
NQT = S // TQ      # 4 q tiles
NKT = S // 128     # 16 k chunks
NDM = D_MODEL // 128  # 16 d_model chunks

# Two-phase Schraudolph exp in bf16-bit space:
#   exp(s) ~= bitcast_bf16(int16(s*A + B1)) + bitcast_bf16(int16(s*A + B2))
# B1/B2 are half a bf16-exponent-period (64) apart and biased by -128 (a
# factor 0.5 each) so the SUM approximates exp(s); the -10.13 centers the
# ripple at zero mean (+-1.7% max, 0.56% rms vs 3% for single-phase).
EXPA = 184.66500854
EXPB1 = 16256.0 - 160.0 - 10.13
EXPB2 = 16256.0 - 96.0 - 10.13

_CACHE = {}
TRACE = False
LAST_RESULT = None


def _build():
    import concourse.mybir as mybir
    import concourse.tile as tile
    from concourse import bacc
    from concourse.masks import make_identity

    F32 = mybir.dt.float32
    F32R = mybir.dt.float32r
    BF16 = mybir.dt.bfloat16
    I16 = mybir.dt.int16

    nc = bacc.Bacc("TRN2", target_bir_lowering=False, num_devices=N_CORES)

    xT = nc.dram_tensor("xT", [D_MODEL, S], BF16, kind="ExternalInput")
    wqT = nc.dram_tensor("wqT", [D_MODEL, RLOC], BF16, kind="ExternalInput")
    wkT = nc.dram_tensor("wkT", [D_MODEL, RLOC], BF16, kind="ExternalInput")
    wvT = nc.dram_tensor("wvT", [D_MODEL, RLOC], BF16, kind="ExternalInput")
    woTs = nc.dram_tensor("woTs", [R, 512], BF16, kind="ExternalInput")
    bq = nc.dram_tensor("bq", [RLOC, 1], F32, kind="ExternalInput")
    bk = nc.dram_tensor("bk", [RLOC, 1], F32, kind="ExternalInput")
    bv = nc.dram_tensor("bv", [RLOC, 1], F32, kind="ExternalInput")
    bo2 = nc.dram_tensor("bo2", [128, 4], F32, kind="ExternalInput")
    outT = nc.dram_tensor("outT", [512, S], F32, kind="ExternalOutput")

    cc_in = [
        nc.dram_tensor(f"cc_in{i}", [RLOC, TQ], BF16, kind="Internal")
        for i in range(NQT)
    ]
    cc_out = [
        nc.dram_tensor(f"cc_out{i}", [R, TQ], BF16, kind="Internal")
        for i in range(NQT)
    ]
    replica_groups = [[0, 1, 2, 3], [4, 5, 6, 7]]

    with tile.TileContext(nc) as tc:
        with (
            tc.tile_pool(name="const", bufs=1) as const,
            tc.tile_pool(name="wpool", bufs=1) as wpool,
            tc.tile_pool(name="xpool", bufs=1) as xpool,
            tc.tile_pool(name="qkv", bufs=1) as qkv,
            tc.tile_pool(name="attnp", bufs=8) as attnp,
            tc.tile_pool(name="expp", bufs=4) as expp,
            tc.tile_pool(name="denp", bufs=2) as denp,
            tc.tile_pool(name="otp", bufs=2) as otp,
            tc.tile_pool(name="agp", bufs=8) as agp,
            tc.tile_pool(name="outp", bufs=2) as outp,
        ):
            # ---- weight / bias / x loads ---------------------------------
            wk_sb = wpool.tile([128, NDM, RLOC], BF16)
            wv_sb = wpool.tile([128, NDM, RLOC], BF16)
            wq_sb = wpool.tile([128, NDM, RLOC], BF16)
            for dm in range(NDM):
                rs = slice(128 * dm, 128 * (dm + 1))
                nc.sync.dma_start(wk_sb[:, dm, :], wkT[rs, :])
                nc.sync.dma_start(wv_sb[:, dm, :], wvT[rs, :])
            bq_sb = const.tile([RLOC, 1], F32)
            bk_sb = const.tile([RLOC, 1], F32)
            bv_sb = const.tile([RLOC, 1], F32)
            bo_sb = const.tile([128, 4], F32)
            nc.sync.dma_start(bk_sb, bk[:])
            nc.sync.dma_start(bv_sb, bv[:])
            nc.sync.dma_start(bq_sb, bq[:])
            nc.sync.dma_start(bo_sb, bo2[:])

            # x resident in SBUF, one tile per d_model chunk so the KV
            # matmuls start as soon as their own chunk lands
            x_sb = [
                xpool.tile([128, S], BF16, name=f"x_sb{dm}") for dm in range(NDM)
            ]
            for dm in range(NDM):
                nc.sync.dma_start(x_sb[dm][:], xT[128 * dm : 128 * (dm + 1), :])

            for dm in range(NDM):
                rs = slice(128 * dm, 128 * (dm + 1))
                nc.sync.dma_start(wq_sb[:, dm, :], wqT[rs, :])
            wo_sb = wpool.tile([128, 4, 4, 128], BF16)
            for rc in range(4):
                for dmt in range(4):
                    nc.sync.dma_start(
                        wo_sb[:, rc, dmt, :],
                        woTs[128 * rc : 128 * (rc + 1), 128 * dmt : 128 * (dmt + 1)],
                    )

            ident = const.tile([128, 128], BF16)
            make_identity(nc, ident[:])

            kt = qkv.tile([RLOC, S], BF16)
            qt = qkv.tile([RLOC, S], BF16)
            vt_bf = qkv.tile([RLOC, S], BF16)
            # [V chunk | ones] stationaries: [128k, kc, h, 64]
            v_aug = qkv.tile([128, NKT, NH, 64], BF16)
            nc.vector.memset(v_aug[:, :, :, 32:64], 1.0)

            # ---- K,V projections (weight-stationary, 4 token tiles) ------
            ps_kv_ctx = tc.tile_pool(name="ps_kv", bufs=8, space="PSUM")
            ps_kv = ps_kv_ctx.__enter__()
            k_ps = [ps_kv.tile([128, TQ], F32, tag="kv", name=f"k_ps{t}") for t in range(NQT)]
            v_ps = [ps_kv.tile([128, TQ], F32, tag="kv", name=f"v_ps{t}") for t in range(NQT)]
            for dm in range(NDM):
                st = dm == 0
                sp = dm == NDM - 1
                for t in range(NQT):
                    nc.tensor.matmul(
                        k_ps[t][:], wk_sb[:, dm, :],
                        x_sb[dm][:, TQ * t : TQ * (t + 1)],
                        start=st, stop=sp,
                    )
                for t in range(NQT):
                    nc.tensor.matmul(
                        v_ps[t][:], wv_sb[:, dm, :],
                        x_sb[dm][:, TQ * t : TQ * (t + 1)],
                        start=st, stop=sp,
                    )
            for t in range(NQT):
                tsl = slice(TQ * t, TQ * (t + 1))
                nc.vector.tensor_scalar_add(kt[:, tsl], k_ps[t][:], bk_sb[:])
                nc.vector.tensor_scalar_add(vt_bf[:, tsl], v_ps[t][:], bv_sb[:])
            ps_kv_ctx.__exit__(None, None, None)

            # ---- Q projection + V transposes -----------------------------
            ps_q_ctx = tc.tile_pool(name="ps_q", bufs=4, space="PSUM")
            ps_tr_ctx = tc.tile_pool(name="ps_tr", bufs=2, space="PSUM")
            ps_q = ps_q_ctx.__enter__()
            ps_tr = ps_tr_ctx.__enter__()
            q_ps = [ps_q.tile([128, TQ], F32, tag="q", name=f"q_ps{t}") for t in range(NQT)]
            for dm in range(NDM):
                st = dm == 0
                sp = dm == NDM - 1
                for t in range(NQT):
                    nc.tensor.matmul(
                        q_ps[t][:], wq_sb[:, dm, :],
                        x_sb[dm][:, TQ * t : TQ * (t + 1)],
                        start=st, stop=sp,
                    )
            for t in range(NQT):
                tsl = slice(TQ * t, TQ * (t + 1))
                nc.vector.tensor_scalar_add(qt[:, tsl], q_ps[t][:], bq_sb[:])
            # V^T -> [tok, chan] chunks, written as 4 per-head slices of v_aug
            for c in range(NKT):
                pst = ps_tr.tile([128, 128], BF16, tag="tr")
                nc.tensor.transpose(
                    pst[:], vt_bf[:, 128 * c : 128 * (c + 1)], ident[:]
                )
                nc.vector.tensor_copy(
                    v_aug[:, c, :, 0:32],
                    pst[:].rearrange("p (h d) -> p h d", h=4),
                )
            ps_tr_ctx.__exit__(None, None, None)
            ps_q_ctx.__exit__(None, None, None)

            # ---- attention -----------------------------------------------
            # psum: 2 score groups (2 banks each) + 2 AV accumulators
            #       + 2 out_proj/spare = 8 banks
            ps_sc_ctx = tc.tile_pool(name="ps_sc", bufs=3, space="PSUM")
            ps_av_ctx = tc.tile_pool(name="ps_av", bufs=2, space="PSUM")
            ps_sc = ps_sc_ctx.__enter__()
            ps_av = ps_av_ctx.__enter__()

            def out_proj(q):
                qsl = slice(TQ * q, TQ * (q + 1))
                ag_t = []
                for rc in range(GROUP):
                    t_ = agp.tile([128, TQ], BF16)
                    nc.sync.dma_start(t_, cc_out[q][128 * rc : 128 * (rc + 1), :])
                    ag_t.append(t_)
                for dmt in range(4):
                    pso2 = ps_sc.tile([128, TQ], F32, tag="sc", name=f"pso2_{q}_{dmt}")
                    for rc in range(GROUP):
                        nc.tensor.matmul(
                            pso2[:], wo_sb[:, rc, dmt, :], ag_t[rc][:],
                            start=(rc == 0), stop=(rc == GROUP - 1),
                        )
                    ob = outp.tile([128, TQ], F32)
                    nc.vector.tensor_scalar_add(
                        ob[:], pso2[:], bo_sb[:, dmt : dmt + 1]
                    )
                    nc.sync.dma_start(outT[128 * dmt : 128 * (dmt + 1), qsl], ob[:])

            for q in range(NQT):
                qsl = slice(TQ * q, TQ * (q + 1))
                av_ab = [
                    ps_av.tile([128, TQ], F32, tag="av", name=f"av_a{q}"),
                    ps_av.tile([128, TQ], F32, tag="av", name=f"av_b{q}"),
                ]
                for kc in range(NKT):
                    st = kc == 0
                    sp = kc == NKT - 1
                    att_pair = []
                    for p in range(2):
                        g = 2 * kc + p
                        pss = ps_sc.tile([128, 2 * TQ], F32, tag="sc")
                        for i in range(2):
                            h = 2 * p + i
                            nc.tensor.matmul(
                                pss[:, TQ * i : TQ * (i + 1)],
                                kt[32 * h : 32 * (h + 1), 128 * kc : 128 * (kc + 1)],
                                qt[32 * h : 32 * (h + 1), qsl],
                                start=True, stop=True,
                                tile_position=(32 * h, 0),
                            )
                        att = attnp.tile([128, 2 * TQ], BF16, tag="at")
                        # split exp: 24/32 of the groups on ACT (exact),
                        # 8/32 on DVE (two-phase Schraudolph bit-trick,
                        # summed on the otherwise idle gpsimd engine)
                        if ((2 * kc + p) * 3) % 4 < 3:
                            nc.scalar.activation(
                                att[:], pss[:], mybir.ActivationFunctionType.Exp
                            )
                        else:
                            e1 = expp.tile([128, 2 * TQ], BF16)
                            e2 = expp.tile([128, 2 * TQ], BF16)
                            nc.vector.tensor_scalar(
                                e1[:].bitcast(I16), pss[:],
                                EXPA, EXPB1,
                                mybir.AluOpType.mult, mybir.AluOpType.add,
                            )
                            nc.vector.tensor_scalar(
                                e2[:].bitcast(I16), pss[:],
                                EXPA, EXPB2,
                                mybir.AluOpType.mult, mybir.AluOpType.add,
                            )
                            nc.gpsimd.tensor_add(att[:], e1[:], e2[:])
                        att_pair.append(att)
                    for h in range(NH):
                        av = av_ab[h // 2]
                        off = 64 * (h % 2)
                        a_ap = att_pair[h // 2][:, TQ * (h % 2) : TQ * (h % 2 + 1)]
                        nc.tensor.matmul(
                            av[off : off + 64, :],
                            v_aug[:, kc, h, :],
                            a_ap,
                            start=st, stop=sp,
                            tile_position=(0, off),
                        )

                # divide AV rows by the denominator rows (32 partitions up)
                ot = otp.tile([128, TQ], BF16)
                for half in range(2):
                    av = av_ab[half]
                    rb = denp.tile([128, TQ], F32)
                    nc.vector.reciprocal_approx_fast(rb[:], av[:])
                    for i in range(2):
                        nc.vector.tensor_mul(
                            ot[64 * half + 32 * i : 64 * half + 32 * (i + 1), :],
                            av[64 * i : 64 * i + 32, :],
                            rb[64 * i + 32 : 64 * (i + 1), :],
                        )
                nc.sync.dma_start(cc_in[q][:], ot[:])
                nc.gpsimd.collective_compute(
                    "AllGather",
                    mybir.AluOpType.bypass,
                    replica_groups=replica_groups,
                    ins=[cc_in[q][:]],
                    outs=[cc_out[q][:]],
                )
                if q >= 1:
                    out_proj(q - 1)
            out_proj(NQT - 1)

            ps_av_ctx.__exit__(None, None, None)
            ps_sc_ctx.__exit__(None, None, None)

    nc.finalize()
    return nc


def _prepare_inputs(x, Wq, bq, Wk, bk, Wv, bv, Wo, bo):
    import ml_dtypes

    bf16 = ml_dtypes.bfloat16
    scale = 1.0 / math.sqrt(D_K)
    x = np.asarray(x, np.float32)
    in_maps = []
    for c in range(N_CORES):
        b, j = divmod(c, GROUP)
        rsl = slice(RLOC * j, RLOC * (j + 1))
        dsl = slice(512 * j, 512 * (j + 1))
        in_maps.append(
            {
                "xT": np.ascontiguousarray(x[b].T).astype(bf16),
                "wqT": np.ascontiguousarray(
                    (np.asarray(Wq, np.float32)[rsl] * scale).T
                ).astype(bf16),
                "wkT": np.ascontiguousarray(
                    np.asarray(Wk, np.float32)[rsl].T
                ).astype(bf16),
                "wvT": np.ascontiguousarray(
                    np.asarray(Wv, np.float32)[rsl].T
                ).astype(bf16),
                "woTs": np.ascontiguousarray(
                    np.asarray(Wo, np.float32)[dsl].T
                ).astype(bf16),
                "bq": (np.asarray(bq)[rsl] * scale).astype(np.float32).reshape(RLOC, 1),
                "bk": np.asarray(bk)[rsl].astype(np.float32).reshape(RLOC, 1),
                "bv": np.asarray(bv)[rsl].astype(np.float32).reshape(RLOC, 1),
                "bo2": np.ascontiguousarray(
                    np.asarray(bo)[dsl].astype(np.float32).reshape(4, 128).T
                ),
            }
        )
    return in_maps


def kernel(x, Wq, bq, Wk, bk, Wv, bv, Wo, bo, mask=None):
    global LAST_RESULT
    from concourse.bass_utils import run_bass_kernel_spmd

    if "nc" not in _CACHE:
        _CACHE["nc"] = _build()
    nc = _CACHE["nc"]

    in_maps = _prepare_inputs(x, Wq, bq, Wk, bk, Wv, bv, Wo, bo)
    res = run_bass_kernel_spmd(
        nc, in_maps, core_ids=list(range(N_CORES)), trace=TRACE
    )
    LAST_RESULT = res
    out = np.empty((B, S, D_MODEL), np.float32)
    for c in range(N_CORES):
        b, j = divmod(c, GROUP)
        out[b, :, 512 * j : 512 * (j + 1)] = res.results[c]["outT"].T
    return out


# revision 12
# speedup vs baseline: 1.1342x; 1.1342x over previous
"""Trainium2 Bass kernel for LowDimProjectedAttention (v2).

Model (reference):
  Q = x @ Wq.T ; K,V likewise  (d_model=2048 -> r=512)
  16 heads of d_k=32, softmax(QK^T/sqrt(32)) @ V, then out_proj r->d_model.
  B=2, S=2048. mask is all-ones (spec fill), dropout p=0, biases zero but
  still applied.

Sharding (8 cores): core c handles batch b=c//4 and the 4 heads j=c%4
(128 of the 512 r-channels, column-parallel QKV). Attention is fully
local. A 4-way AllGather per q tile rebuilds attn_out (bf16), then each
core computes a 512-wide slice of d_model (column-parallel out_proj).

v2 structure (vs v1):
  - x and all weights are bf16 (host-cast): halves DMA and SBUF.
  - K,V projections run weight-stationary (dm-outer, 4 token tiles inner)
    so consecutive matmuls reuse the stationary operand; Q likewise.
  - Softmax denominator is folded into the AV matmul: the stationary is
    [V_chunk | ones] (64 cols), so out rows 0-31 are AV and 32-63 the
    denominator broadcast - the separate ones-matmul stream is gone.
  - exp is split between ACT (exact, bf16 out) and DVE (Schraudolph
    int16 bit-trick approximating exp in bf16), removing the Scalar
    engine as the per-tile bottleneck.
  - out_proj for tile q is emitted after attention of tile q+1 so the
    AllGather never head-of-line blocks the PE queue.
"""

import math

import numpy as np

B = 2
S = 2048
D_MODEL = 2048
R = 512
N_HEADS = 16
D_K = 32
N_CORES = 8
GROUP = 4          # cores per batch group
RLOC = 128         # r-channels per core (4 heads x 32)
NH = 4             # heads per core
TQ = 512           # q tile size
NQT = S // TQ      # 4 q tiles
NKT = S // 128     # 16 k chunks
NDM = D_MODEL // 128  # 16 d_model chunks

# Two-phase Schraudolph exp in bf16-bit space:
#   exp(s) ~= bitcast_bf16(int16(s*A + B1)) + bitcast_bf16(int16(s*A + B2))
# B1/B2 are half a bf16-exponent-period (64) apart and biased by -128 (a
# factor 0.5 each) so the SUM approximates exp(s); the -10.13 centers the
# ripple at zero mean (+-1.7% max, 0.56% rms vs 3% for single-phase).
EXPA = 184.66500854
EXPB1 = 16256.0 - 160.0 - 10.13
EXPB2 = 16256.0 - 96.0 - 10.13
AV_DELAY = 3      # AV consumption trails scores/exp by this many groups

_CACHE = {}
TRACE = False
LAST_RESULT = None


def _build():
    import concourse.mybir as mybir
    import concourse.tile as tile
    from concourse import bacc
    from concourse.masks import make_identity

    F32 = mybir.dt.float32
    F32R = mybir.dt.float32r
    BF16 = mybir.dt.bfloat16
    I16 = mybir.dt.int16

    nc = bacc.Bacc("TRN2", target_bir_lowering=False, num_devices=N_CORES)

    xT = nc.dram_tensor("xT", [D_MODEL, S], BF16, kind="ExternalInput")
    wqT = nc.dram_tensor("wqT", [D_MODEL, RLOC], BF16, kind="ExternalInput")
    wkT = nc.dram_tensor("wkT", [D_MODEL, RLOC], BF16, kind="ExternalInput")
    wvT = nc.dram_tensor("wvT", [D_MODEL, RLOC], BF16, kind="ExternalInput")
    woTs = nc.dram_tensor("woTs", [R, 512], BF16, kind="ExternalInput")
    bq = nc.dram_tensor("bq", [RLOC, 1], F32, kind="ExternalInput")
    bk = nc.dram_tensor("bk", [RLOC, 1], F32, kind="ExternalInput")
    bv = nc.dram_tensor("bv", [RLOC, 1], F32, kind="ExternalInput")
    bo2 = nc.dram_tensor("bo2", [128, 4], F32, kind="ExternalInput")
    outT = nc.dram_tensor("outT", [512, S], F32, kind="ExternalOutput")

    cc_in = [
        nc.dram_tensor(f"cc_in{i}", [RLOC, TQ], BF16, kind="Internal")
        for i in range(NQT)
    ]
    cc_out = [
        nc.dram_tensor(f"cc_out{i}", [R, TQ], BF16, kind="Internal")
        for i in range(NQT)
    ]
    replica_groups = [[0, 1, 2, 3], [4, 5, 6, 7]]

    with tile.TileContext(nc) as tc:
        with (
            tc.tile_pool(name="const", bufs=1) as const,
            tc.tile_pool(name="wpool", bufs=1) as wpool,
            tc.tile_pool(name="xpool", bufs=1) as xpool,
            tc.tile_pool(name="qkv", bufs=1) as qkv,
            tc.tile_pool(name="attnp", bufs=8) as attnp,
            tc.tile_pool(name="expp", bufs=4) as expp,
            tc.tile_pool(name="denp", bufs=2) as denp,
            tc.tile_pool(name="otp", bufs=2) as otp,
            tc.tile_pool(name="agp", bufs=8) as agp,
            tc.tile_pool(name="outp", bufs=2) as outp,
        ):
            # ---- weight / bias / x loads ---------------------------------
            # interleave the DMA *issue* order (the issuing engine pays
            # ~0.6us per dma_start, so a long run of weight DMAs ahead of x
            # delays the first matmul by tens of us): x on the scalar
            # queue, wk/wv on sync, both dm-ascending.
            wk_sb = wpool.tile([128, NDM, RLOC], BF16)
            wv_sb = wpool.tile([128, NDM, RLOC], BF16)
            wq_sb = wpool.tile([128, NDM, RLOC], BF16)
            x_sb = [
                xpool.tile([128, S], BF16, name=f"x_sb{dm}") for dm in range(NDM)
            ]
            for dm in range(NDM):
                rs = slice(128 * dm, 128 * (dm + 1))
                nc.scalar.dma_start(x_sb[dm][:], xT[rs, :])
                nc.sync.dma_start(wk_sb[:, dm, :], wkT[rs, :])
                nc.sync.dma_start(wv_sb[:, dm, :], wvT[rs, :])
            bq_sb = const.tile([RLOC, 1], F32)
            bk_sb = const.tile([RLOC, 1], F32)
            bv_sb = const.tile([RLOC, 1], F32)
            bo_sb = const.tile([128, 4], F32)
            nc.gpsimd.dma_start(bk_sb, bk[:])
            nc.gpsimd.dma_start(bv_sb, bv[:])
            nc.gpsimd.dma_start(bq_sb, bq[:])
            nc.gpsimd.dma_start(bo_sb, bo2[:])
            for dm in range(NDM):
                rs = slice(128 * dm, 128 * (dm + 1))
                nc.scalar.dma_start(wq_sb[:, dm, :], wqT[rs, :])
            wo_sb = wpool.tile([128, 4, 4, 128], BF16)
            for rc in range(4):
                for dmt in range(4):
                    nc.sync.dma_start(
                        wo_sb[:, rc, dmt, :],
                        woTs[128 * rc : 128 * (rc + 1), 128 * dmt : 128 * (dmt + 1)],
                    )

            ident = const.tile([128, 128], BF16)
            make_identity(nc, ident[:])

            kt = qkv.tile([RLOC, S], BF16)
            qt = qkv.tile([RLOC, S], BF16)
            vt_bf = qkv.tile([RLOC, S], BF16)
            # [V chunk | ones] stationaries: bf16 per-kc for the DVE-exp
            # groups, fp8 DoubleRow-interleaved per kc-pair for ACT groups
            v_aug = qkv.tile([128, NKT, NH, 64], BF16)
            nc.vector.memset(v_aug[:, :, :, 32:64], 1.0)

            # ---- K,V projections (weight-stationary, 4 token tiles) ------
            ps_kv_ctx = tc.tile_pool(name="ps_kv", bufs=8, space="PSUM")
            ps_kv = ps_kv_ctx.__enter__()
            k_ps = [ps_kv.tile([128, TQ], F32, tag="kv", name=f"k_ps{t}") for t in range(NQT)]
            v_ps = [ps_kv.tile([128, TQ], F32, tag="kv", name=f"v_ps{t}") for t in range(NQT)]
            for dm in range(NDM):
                st = dm == 0
                sp = dm == NDM - 1
                for t in range(NQT):
                    nc.tensor.matmul(
                        k_ps[t][:], wk_sb[:, dm, :],
                        x_sb[dm][:, TQ * t : TQ * (t + 1)],
                        start=st, stop=sp,
                    )
                for t in range(NQT):
                    nc.tensor.matmul(
                        v_ps[t][:], wv_sb[:, dm, :],
                        x_sb[dm][:, TQ * t : TQ * (t + 1)],
                        start=st, stop=sp,
                    )
            for t in range(NQT):
                tsl = slice(TQ * t, TQ * (t + 1))
                nc.vector.tensor_scalar_add(kt[:, tsl], k_ps[t][:], bk_sb[:])
                nc.vector.tensor_scalar_add(vt_bf[:, tsl], v_ps[t][:], bv_sb[:])
            ps_kv_ctx.__exit__(None, None, None)

            # ---- Q projection + V transposes -----------------------------
            ps_q_ctx = tc.tile_pool(name="ps_q", bufs=4, space="PSUM")
            ps_tr_ctx = tc.tile_pool(name="ps_tr", bufs=2, space="PSUM")
            ps_q = ps_q_ctx.__enter__()
            ps_tr = ps_tr_ctx.__enter__()
            q_ps = [ps_q.tile([128, TQ], F32, tag="q", name=f"q_ps{t}") for t in range(NQT)]
            for dm in range(NDM):
                st = dm == 0
                sp = dm == NDM - 1
                for t in range(NQT):
                    nc.tensor.matmul(
                        q_ps[t][:], wq_sb[:, dm, :],
                        x_sb[dm][:, TQ * t : TQ * (t + 1)],
                        start=st, stop=sp,
                    )
            for t in range(NQT):
                tsl = slice(TQ * t, TQ * (t + 1))
                nc.vector.tensor_scalar_add(qt[:, tsl], q_ps[t][:], bq_sb[:])
            # V^T -> [tok, chan] chunks, written as 4 per-head slices of v_aug
            for c in range(NKT):
                pst = ps_tr.tile([128, 128], BF16, tag="tr")
                nc.tensor.transpose(
                    pst[:], vt_bf[:, 128 * c : 128 * (c + 1)], ident[:]
                )
                nc.vector.tensor_copy(
                    v_aug[:, c, :, 0:32],
                    pst[:].rearrange("p (h d) -> p h d", h=4),
                )
            ps_tr_ctx.__exit__(None, None, None)
            ps_q_ctx.__exit__(None, None, None)

            # ---- attention -----------------------------------------------
            # psum: 2 score groups (2 banks each) + 2 AV accumulators
            #       + 2 out_proj/spare = 8 banks
            ps_sc_ctx = tc.tile_pool(name="ps_sc", bufs=3, space="PSUM")
            ps_av_ctx = tc.tile_pool(name="ps_av", bufs=2, space="PSUM")
            ps_sc = ps_sc_ctx.__enter__()
            ps_av = ps_av_ctx.__enter__()

            def out_proj(q):
                qsl = slice(TQ * q, TQ * (q + 1))
                ag_t = []
                for rc in range(GROUP):
                    t_ = agp.tile([128, TQ], BF16)
                    nc.sync.dma_start(t_, cc_out[q][128 * rc : 128 * (rc + 1), :])
                    ag_t.append(t_)
                for dmt in range(4):
                    pso2 = ps_sc.tile([128, TQ], F32, tag="sc", name=f"pso2_{q}_{dmt}")
                    for rc in range(GROUP):
                        nc.tensor.matmul(
                            pso2[:], wo_sb[:, rc, dmt, :], ag_t[rc][:],
                            start=(rc == 0), stop=(rc == GROUP - 1),
                        )
                    ob = outp.tile([128, TQ], F32)
                    nc.vector.tensor_scalar_add(
                        ob[:], pso2[:], bo_sb[:, dmt : dmt + 1]
                    )
                    nc.sync.dma_start(outT[128 * dmt : 128 * (dmt + 1), qsl], ob[:])

            for q in range(NQT):
                qsl = slice(TQ * q, TQ * (q + 1))
                av_ab = [
                    ps_av.tile([128, TQ], F32, tag="av", name=f"av_a{q}"),
                    ps_av.tile([128, TQ], F32, tag="av", name=f"av_b{q}"),
                ]
                # group g = (kc pair, head): scores for k chunks 2*kcp and
                # 2*kcp+1 of head h share one 2-bank psum tile. exp runs on
                # ACT (fp8 out, DoubleRow AV) for 3 of 4 groups and on DVE
                # (two-phase Schraudolph, bf16 AV) for the rest. The AV
                # stream trails by AV_DELAY groups so exp latency never
                # stalls the PE.
                NG = 2 * NKT          # 32 groups of [128, 1024]
                att_of = {}
                for gi in range(NG + AV_DELAY):
                    if gi < NG:
                        kc, p = divmod(gi, 2)
                        pss = ps_sc.tile([128, 2 * TQ], F32, tag="sc",
                                         name=f"pss{q}_{gi}")
                        for i in range(2):
                            h = 2 * p + i
                            nc.tensor.matmul(
                                pss[:, TQ * i : TQ * (i + 1)],
                                kt[32 * h : 32 * (h + 1), 128 * kc : 128 * (kc + 1)],
                                qt[32 * h : 32 * (h + 1), qsl],
                                start=True, stop=True,
                                tile_position=(32 * h, 0),
                            )
                        att = attnp.tile([128, 2 * TQ], BF16, tag="at",
                                         name=f"att_{q}_{gi}")
                        if (gi * 3) % 4 < 3:
                            nc.scalar.activation(
                                att[:], pss[:], mybir.ActivationFunctionType.Exp
                            )
                        else:
                            e1 = expp.tile([128, 2 * TQ], BF16)
                            e2 = expp.tile([128, 2 * TQ], BF16)
                            nc.vector.tensor_scalar(
                                e1[:].bitcast(I16), pss[:],
                                EXPA, EXPB1,
                                mybir.AluOpType.mult, mybir.AluOpType.add,
                            )
                            nc.vector.tensor_scalar(
                                e2[:].bitcast(I16), pss[:],
                                EXPA, EXPB2,
                                mybir.AluOpType.mult, mybir.AluOpType.add,
                            )
                            nc.gpsimd.tensor_add(att[:], e1[:], e2[:])
                        att_of[gi] = att
                    ai = gi - AV_DELAY
                    if ai >= 0:
                        kc, p = divmod(ai, 2)
                        st = kc == 0
                        sp = kc == NKT - 1
                        att = att_of.pop(ai)
                        for i in range(2):
                            h = 2 * p + i
                            nc.tensor.matmul(
                                av_ab[p][64 * i : 64 * (i + 1), :],
                                v_aug[:, kc, h, :],
                                att[:, TQ * i : TQ * (i + 1)],
                                start=st, stop=sp,
                                tile_position=(0, 64 * i),
                            )

                # divide AV rows by the denominator rows (32 partitions up)
                ot = otp.tile([128, TQ], BF16)
                for half in range(2):
                    av = av_ab[half]
                    rb = denp.tile([128, TQ], F32)
                    nc.vector.reciprocal_approx_fast(rb[:], av[:])
                    for i in range(2):
                        nc.vector.tensor_mul(
                            ot[64 * half + 32 * i : 64 * half + 32 * (i + 1), :],
                            av[64 * i : 64 * i + 32, :],
                            rb[64 * i + 32 : 64 * (i + 1), :],
                        )
                nc.sync.dma_start(cc_in[q][:], ot[:])
                nc.gpsimd.collective_compute(
                    "AllGather",
                    mybir.AluOpType.bypass,
                    replica_groups=replica_groups,
                    ins=[cc_in[q][:]],
                    outs=[cc_out[q][:]],
                )
                if q >= 1:
                    out_proj(q - 1)
            out_proj(NQT - 1)

            ps_av_ctx.__exit__(None, None, None)
            ps_sc_ctx.__exit__(None, None, None)

    nc.finalize()
    return nc


def _prepare_inputs(x, Wq, bq, Wk, bk, Wv, bv, Wo, bo):
    import ml_dtypes

    bf16 = ml_dtypes.bfloat16
    scale = 1.0 / math.sqrt(D_K)
    x = np.asarray(x, np.float32)
    in_maps = []
    for c in range(N_CORES):
        b, j = divmod(c, GROUP)
        rsl = slice(RLOC * j, RLOC * (j + 1))
        dsl = slice(512 * j, 512 * (j + 1))
        in_maps.append(
            {
                "xT": np.ascontiguousarray(x[b].T).astype(bf16),
                "wqT": np.ascontiguousarray(
                    (np.asarray(Wq, np.float32)[rsl] * scale).T
                ).astype(bf16),
                "wkT": np.ascontiguousarray(
                    np.asarray(Wk, np.float32)[rsl].T
                ).astype(bf16),
                "wvT": np.ascontiguousarray(
                    np.asarray(Wv, np.float32)[rsl].T
                ).astype(bf16),
                "woTs": np.ascontiguousarray(
                    np.asarray(Wo, np.float32)[dsl].T
                ).astype(bf16),
                "bq": (np.asarray(bq)[rsl] * scale).astype(np.float32).reshape(RLOC, 1),
                "bk": np.asarray(bk)[rsl].astype(np.float32).reshape(RLOC, 1),
                "bv": np.asarray(bv)[rsl].astype(np.float32).reshape(RLOC, 1),
                "bo2": np.ascontiguousarray(
                    np.asarray(bo)[dsl].astype(np.float32).reshape(4, 128).T
                ),
            }
        )
    return in_maps


def kernel(x, Wq, bq, Wk, bk, Wv, bv, Wo, bo, mask=None):
    global LAST_RESULT
    from concourse.bass_utils import run_bass_kernel_spmd

    if "nc" not in _CACHE:
        _CACHE["nc"] = _build()
    nc = _CACHE["nc"]

    in_maps = _prepare_inputs(x, Wq, bq, Wk, bk, Wv, bv, Wo, bo)
    res = run_bass_kernel_spmd(
        nc, in_maps, core_ids=list(range(N_CORES)), trace=TRACE
    )
    LAST_RESULT = res
    out = np.empty((B, S, D_MODEL), np.float32)
    for c in range(N_CORES):
        b, j = divmod(c, GROUP)
        out[b, :, 512 * j : 512 * (j + 1)] = res.results[c]["outT"].T
    return out


# revision 13
# speedup vs baseline: 1.2471x; 1.0996x over previous
"""Trainium2 Bass kernel for LowDimProjectedAttention (v2).

Model (reference):
  Q = x @ Wq.T ; K,V likewise  (d_model=2048 -> r=512)
  16 heads of d_k=32, softmax(QK^T/sqrt(32)) @ V, then out_proj r->d_model.
  B=2, S=2048. mask is all-ones (spec fill), dropout p=0, biases zero but
  still applied.

Sharding (8 cores): core c handles batch b=c//4 and the 4 heads j=c%4
(128 of the 512 r-channels, column-parallel QKV). Attention is fully
local. A 4-way AllGather per q tile rebuilds attn_out (bf16), then each
core computes a 512-wide slice of d_model (column-parallel out_proj).

v2 structure (vs v1):
  - x and all weights are bf16 (host-cast): halves DMA and SBUF.
  - K,V projections run weight-stationary (dm-outer, 4 token tiles inner)
    so consecutive matmuls reuse the stationary operand; Q likewise.
  - Softmax denominator is folded into the AV matmul: the stationary is
    [V_chunk | ones] (64 cols), so out rows 0-31 are AV and 32-63 the
    denominator broadcast - the separate ones-matmul stream is gone.
  - exp is split between ACT (exact, bf16 out) and DVE (Schraudolph
    int16 bit-trick approximating exp in bf16), removing the Scalar
    engine as the per-tile bottleneck.
  - out_proj for tile q is emitted after attention of tile q+1 so the
    AllGather never head-of-line blocks the PE queue.
"""

import math

import numpy as np

B = 2
S = 2048
D_MODEL = 2048
R = 512
N_HEADS = 16
D_K = 32
N_CORES = 8
GROUP = 4          # cores per batch group
RLOC = 128         # r-channels per core (4 heads x 32)
NH = 4             # heads per core
TQ = 512           # q tile size
NQT = S // TQ      # 4 q tiles
NKT = S // 128     # 16 k chunks
NDM = D_MODEL // 128  # 16 d_model chunks

# Two-phase Schraudolph exp in bf16-bit space:
#   exp(s) ~= bitcast_bf16(int16(s*A + B1)) + bitcast_bf16(int16(s*A + B2))
# B1/B2 are half a bf16-exponent-period (64) apart and biased by -128 (a
# factor 0.5 each) so the SUM approximates exp(s); the -10.13 centers the
# ripple at zero mean (+-1.7% max, 0.56% rms vs 3% for single-phase).
EXPA = 184.66500854
EXPB1 = 16256.0 - 160.0 - 10.13
EXPB2 = 16256.0 - 96.0 - 10.13
AV_DELAY = 6      # AV consumption trails scores/exp by this many groups

_CACHE = {}
TRACE = False
LAST_RESULT = None


def _build():
    import concourse.mybir as mybir
    import concourse.tile as tile
    from concourse import bacc
    from concourse.masks import make_identity

    F32 = mybir.dt.float32
    F32R = mybir.dt.float32r
    BF16 = mybir.dt.bfloat16
    I16 = mybir.dt.int16

    nc = bacc.Bacc("TRN2", target_bir_lowering=False, num_devices=N_CORES)

    xT = nc.dram_tensor("xT", [D_MODEL, S], BF16, kind="ExternalInput")
    wqT = nc.dram_tensor("wqT", [D_MODEL, RLOC], BF16, kind="ExternalInput")
    wkT = nc.dram_tensor("wkT", [D_MODEL, RLOC], BF16, kind="ExternalInput")
    wvT = nc.dram_tensor("wvT", [D_MODEL, RLOC], BF16, kind="ExternalInput")
    woTs = nc.dram_tensor("woTs", [R, 512], BF16, kind="ExternalInput")
    bq = nc.dram_tensor("bq", [RLOC, 1], F32, kind="ExternalInput")
    bk = nc.dram_tensor("bk", [RLOC, 1], F32, kind="ExternalInput")
    bv = nc.dram_tensor("bv", [RLOC, 1], F32, kind="ExternalInput")
    bo2 = nc.dram_tensor("bo2", [128, 4], F32, kind="ExternalInput")
    outT = nc.dram_tensor("outT", [512, S], F32, kind="ExternalOutput")

    cc_in = [
        nc.dram_tensor(f"cc_in{i}", [RLOC, TQ], BF16, kind="Internal")
        for i in range(NQT)
    ]
    cc_out = [
        nc.dram_tensor(f"cc_out{i}", [R, TQ], BF16, kind="Internal")
        for i in range(NQT)
    ]
    replica_groups = [[0, 1, 2, 3], [4, 5, 6, 7]]

    with tile.TileContext(nc) as tc:
        with (
            tc.tile_pool(name="const", bufs=1) as const,
            tc.tile_pool(name="wpool", bufs=1) as wpool,
            tc.tile_pool(name="xpool", bufs=1) as xpool,
            tc.tile_pool(name="qkv", bufs=1) as qkv,
            tc.tile_pool(name="attnp", bufs=8) as attnp,
            tc.tile_pool(name="expp", bufs=4) as expp,
            tc.tile_pool(name="denp", bufs=2) as denp,
            tc.tile_pool(name="otp", bufs=2) as otp,
            tc.tile_pool(name="agp", bufs=8) as agp,
            tc.tile_pool(name="outp", bufs=2) as outp,
        ):
            # ---- weight / bias / x loads ---------------------------------
            # interleave the DMA *issue* order (the issuing engine pays
            # ~0.6us per dma_start, so a long run of weight DMAs ahead of x
            # delays the first matmul by tens of us): x on the scalar
            # queue, wk/wv on sync, both dm-ascending.
            wk_sb = [
                wpool.tile([128, RLOC], BF16, name=f"wk_sb{dm}")
                for dm in range(NDM)
            ]
            wv_sb = [
                wpool.tile([128, RLOC], BF16, name=f"wv_sb{dm}")
                for dm in range(NDM)
            ]
            wq_sb = [
                wpool.tile([128, RLOC], BF16, name=f"wq_sb{dm}")
                for dm in range(NDM)
            ]
            x_sb = [
                xpool.tile([128, S], BF16, name=f"x_sb{dm}") for dm in range(NDM)
            ]
            for dm in range(NDM):
                rs = slice(128 * dm, 128 * (dm + 1))
                nc.scalar.dma_start(x_sb[dm][:], xT[rs, :])
                nc.sync.dma_start(wk_sb[dm][:], wkT[rs, :])
                nc.sync.dma_start(wv_sb[dm][:], wvT[rs, :])
            bq_sb = const.tile([RLOC, 1], F32)
            bk_sb = const.tile([RLOC, 1], F32)
            bv_sb = const.tile([RLOC, 1], F32)
            bo_sb = const.tile([128, 4], F32)
            nc.gpsimd.dma_start(bk_sb, bk[:])
            nc.gpsimd.dma_start(bv_sb, bv[:])
            nc.gpsimd.dma_start(bq_sb, bq[:])
            nc.gpsimd.dma_start(bo_sb, bo2[:])
            for dm in range(NDM):
                rs = slice(128 * dm, 128 * (dm + 1))
                nc.scalar.dma_start(wq_sb[dm][:], wqT[rs, :])
            wo_sb = wpool.tile([128, 4, 4, 128], BF16)
            for rc in range(4):
                for dmt in range(4):
                    nc.sync.dma_start(
                        wo_sb[:, rc, dmt, :],
                        woTs[128 * rc : 128 * (rc + 1), 128 * dmt : 128 * (dmt + 1)],
                    )

            ident = const.tile([128, 128], BF16)
            make_identity(nc, ident[:])

            kt = qkv.tile([RLOC, S], BF16)
            qt = qkv.tile([RLOC, S], BF16)
            vt_bf = qkv.tile([RLOC, S], BF16)
            # [V chunk | ones] stationaries: bf16 per-kc for the DVE-exp
            # groups, fp8 DoubleRow-interleaved per kc-pair for ACT groups
            v_aug = qkv.tile([128, NKT, NH, 64], BF16)
            nc.vector.memset(v_aug[:, :, :, 32:64], 1.0)

            # ---- K,V projections (weight-stationary, 4 token tiles) ------
            ps_kv_ctx = tc.tile_pool(name="ps_kv", bufs=8, space="PSUM")
            ps_kv = ps_kv_ctx.__enter__()
            k_ps = [ps_kv.tile([128, TQ], F32, tag="kv", name=f"k_ps{t}") for t in range(NQT)]
            v_ps = [ps_kv.tile([128, TQ], F32, tag="kv", name=f"v_ps{t}") for t in range(NQT)]
            for dm in range(NDM):
                st = dm == 0
                sp = dm == NDM - 1
                for t in range(NQT):
                    nc.tensor.matmul(
                        k_ps[t][:], wk_sb[dm][:],
                        x_sb[dm][:, TQ * t : TQ * (t + 1)],
                        start=st, stop=sp,
                    )
                for t in range(NQT):
                    nc.tensor.matmul(
                        v_ps[t][:], wv_sb[dm][:],
                        x_sb[dm][:, TQ * t : TQ * (t + 1)],
                        start=st, stop=sp,
                    )
            for t in range(NQT):
                tsl = slice(TQ * t, TQ * (t + 1))
                nc.vector.tensor_scalar_add(kt[:, tsl], k_ps[t][:], bk_sb[:])
                nc.vector.tensor_scalar_add(vt_bf[:, tsl], v_ps[t][:], bv_sb[:])
            ps_kv_ctx.__exit__(None, None, None)

            # ---- Q projection + V transposes -----------------------------
            ps_q_ctx = tc.tile_pool(name="ps_q", bufs=4, space="PSUM")
            ps_tr_ctx = tc.tile_pool(name="ps_tr", bufs=2, space="PSUM")
            ps_q = ps_q_ctx.__enter__()
            ps_tr = ps_tr_ctx.__enter__()
            q_ps = [ps_q.tile([128, TQ], F32, tag="q", name=f"q_ps{t}") for t in range(NQT)]
            for dm in range(NDM):
                st = dm == 0
                sp = dm == NDM - 1
                for t in range(NQT):
                    nc.tensor.matmul(
                        q_ps[t][:], wq_sb[dm][:],
                        x_sb[dm][:, TQ * t : TQ * (t + 1)],
                        start=st, stop=sp,
                    )
            for t in range(NQT):
                tsl = slice(TQ * t, TQ * (t + 1))
                nc.vector.tensor_scalar_add(qt[:, tsl], q_ps[t][:], bq_sb[:])
            # V^T -> [tok, chan] chunks, written as 4 per-head slices of v_aug
            for c in range(NKT):
                pst = ps_tr.tile([128, 128], BF16, tag="tr")
                nc.tensor.transpose(
                    pst[:], vt_bf[:, 128 * c : 128 * (c + 1)], ident[:]
                )
                nc.vector.tensor_copy(
                    v_aug[:, c, :, 0:32],
                    pst[:].rearrange("p (h d) -> p h d", h=4),
                )
            ps_tr_ctx.__exit__(None, None, None)
            ps_q_ctx.__exit__(None, None, None)

            # ---- attention -----------------------------------------------
            # psum: 2 score groups (2 banks each) + 2 AV accumulators
            #       + 2 out_proj/spare = 8 banks
            ps_sc_ctx = tc.tile_pool(name="ps_sc", bufs=3, space="PSUM")
            ps_av_ctx = tc.tile_pool(name="ps_av", bufs=2, space="PSUM")
            ps_sc = ps_sc_ctx.__enter__()
            ps_av = ps_av_ctx.__enter__()

            def out_proj(q):
                qsl = slice(TQ * q, TQ * (q + 1))
                ag_t = []
                for rc in range(GROUP):
                    t_ = agp.tile([128, TQ], BF16)
                    nc.sync.dma_start(t_, cc_out[q][128 * rc : 128 * (rc + 1), :])
                    ag_t.append(t_)
                for dmt in range(4):
                    pso2 = ps_sc.tile([128, TQ], F32, tag="sc", name=f"pso2_{q}_{dmt}")
                    for rc in range(GROUP):
                        nc.tensor.matmul(
                            pso2[:], wo_sb[:, rc, dmt, :], ag_t[rc][:],
                            start=(rc == 0), stop=(rc == GROUP - 1),
                        )
                    ob = outp.tile([128, TQ], F32)
                    nc.vector.tensor_scalar_add(
                        ob[:], pso2[:], bo_sb[:, dmt : dmt + 1]
                    )
                    nc.sync.dma_start(outT[128 * dmt : 128 * (dmt + 1), qsl], ob[:])

            for q in range(NQT):
                qsl = slice(TQ * q, TQ * (q + 1))
                av_ab = [
                    ps_av.tile([128, TQ], F32, tag="av", name=f"av_a{q}"),
                    ps_av.tile([128, TQ], F32, tag="av", name=f"av_b{q}"),
                ]
                # group g = (kc pair, head): scores for k chunks 2*kcp and
                # 2*kcp+1 of head h share one 2-bank psum tile. exp runs on
                # ACT (fp8 out, DoubleRow AV) for 3 of 4 groups and on DVE
                # (two-phase Schraudolph, bf16 AV) for the rest. The AV
                # stream trails by AV_DELAY groups so exp latency never
                # stalls the PE.
                NG = 2 * NKT          # 32 groups of [128, 1024]
                att_of = {}
                for gi in range(NG + AV_DELAY):
                    if gi < NG:
                        kc, p = divmod(gi, 2)
                        pss = ps_sc.tile([128, 2 * TQ], F32, tag="sc",
                                         name=f"pss{q}_{gi}")
                        for i in range(2):
                            h = 2 * p + i
                            nc.tensor.matmul(
                                pss[:, TQ * i : TQ * (i + 1)],
                                kt[32 * h : 32 * (h + 1), 128 * kc : 128 * (kc + 1)],
                                qt[32 * h : 32 * (h + 1), qsl],
                                start=True, stop=True,
                                tile_position=(32 * h, 0),
                            )
                        att = attnp.tile([128, 2 * TQ], BF16, tag="at",
                                         name=f"att_{q}_{gi}")
                        if (gi * 3) % 4 < 3:
                            nc.scalar.activation(
                                att[:], pss[:], mybir.ActivationFunctionType.Exp
                            )
                        else:
                            e1 = expp.tile([128, 2 * TQ], BF16)
                            e2 = expp.tile([128, 2 * TQ], BF16)
                            nc.vector.tensor_scalar(
                                e1[:].bitcast(I16), pss[:],
                                EXPA, EXPB1,
                                mybir.AluOpType.mult, mybir.AluOpType.add,
                            )
                            nc.vector.tensor_scalar(
                                e2[:].bitcast(I16), pss[:],
                                EXPA, EXPB2,
                                mybir.AluOpType.mult, mybir.AluOpType.add,
                            )
                            nc.vector.tensor_add(att[:], e1[:], e2[:])
                        att_of[gi] = att
                    ai = gi - AV_DELAY
                    if ai >= 0:
                        kc, p = divmod(ai, 2)
                        st = kc == 0
                        sp = kc == NKT - 1
                        att = att_of.pop(ai)
                        for i in range(2):
                            h = 2 * p + i
                            nc.tensor.matmul(
                                av_ab[p][64 * i : 64 * (i + 1), :],
                                v_aug[:, kc, h, :],
                                att[:, TQ * i : TQ * (i + 1)],
                                start=st, stop=sp,
                                tile_position=(0, 64 * i),
                            )

                # divide AV rows by the denominator rows (32 partitions up)
                ot = otp.tile([128, TQ], BF16)
                for half in range(2):
                    av = av_ab[half]
                    rb = denp.tile([128, TQ], F32)
                    nc.vector.reciprocal_approx_fast(rb[:], av[:])
                    for i in range(2):
                        nc.vector.tensor_mul(
                            ot[64 * half + 32 * i : 64 * half + 32 * (i + 1), :],
                            av[64 * i : 64 * i + 32, :],
                            rb[64 * i + 32 : 64 * (i + 1), :],
                        )
                nc.sync.dma_start(cc_in[q][:], ot[:])
                nc.gpsimd.collective_compute(
                    "AllGather",
                    mybir.AluOpType.bypass,
                    replica_groups=replica_groups,
                    ins=[cc_in[q][:]],
                    outs=[cc_out[q][:]],
                )
                if q >= 1:
                    out_proj(q - 1)
            out_proj(NQT - 1)

            ps_av_ctx.__exit__(None, None, None)
            ps_sc_ctx.__exit__(None, None, None)

    nc.finalize()
    return nc


def _prepare_inputs(x, Wq, bq, Wk, bk, Wv, bv, Wo, bo):
    import ml_dtypes

    bf16 = ml_dtypes.bfloat16
    scale = 1.0 / math.sqrt(D_K)
    x = np.asarray(x, np.float32)
    in_maps = []
    for c in range(N_CORES):
        b, j = divmod(c, GROUP)
        rsl = slice(RLOC * j, RLOC * (j + 1))
        dsl = slice(512 * j, 512 * (j + 1))
        in_maps.append(
            {
                "xT": np.ascontiguousarray(x[b].T).astype(bf16),
                "wqT": np.ascontiguousarray(
                    (np.asarray(Wq, np.float32)[rsl] * scale).T
                ).astype(bf16),
                "wkT": np.ascontiguousarray(
                    np.asarray(Wk, np.float32)[rsl].T
                ).astype(bf16),
                "wvT": np.ascontiguousarray(
                    np.asarray(Wv, np.float32)[rsl].T
                ).astype(bf16),
                "woTs": np.ascontiguousarray(
                    np.asarray(Wo, np.float32)[dsl].T
                ).astype(bf16),
                "bq": (np.asarray(bq)[rsl] * scale).astype(np.float32).reshape(RLOC, 1),
                "bk": np.asarray(bk)[rsl].astype(np.float32).reshape(RLOC, 1),
                "bv": np.asarray(bv)[rsl].astype(np.float32).reshape(RLOC, 1),
                "bo2": np.ascontiguousarray(
                    np.asarray(bo)[dsl].astype(np.float32).reshape(4, 128).T
                ),
            }
        )
    return in_maps


def kernel(x, Wq, bq, Wk, bk, Wv, bv, Wo, bo, mask=None):
    global LAST_RESULT
    from concourse.bass_utils import run_bass_kernel_spmd

    if "nc" not in _CACHE:
        _CACHE["nc"] = _build()
    nc = _CACHE["nc"]

    in_maps = _prepare_inputs(x, Wq, bq, Wk, bk, Wv, bv, Wo, bo)
    res = run_bass_kernel_spmd(
        nc, in_maps, core_ids=list(range(N_CORES)), trace=TRACE
    )
    LAST_RESULT = res
    out = np.empty((B, S, D_MODEL), np.float32)
    for c in range(N_CORES):
        b, j = divmod(c, GROUP)
        out[b, :, 512 * j : 512 * (j + 1)] = res.results[c]["outT"].T
    return out


# revision 14
# speedup vs baseline: 1.3323x; 1.0683x over previous
"""Trainium2 Bass kernel for LowDimProjectedAttention (v2).

Model (reference):
  Q = x @ Wq.T ; K,V likewise  (d_model=2048 -> r=512)
  16 heads of d_k=32, softmax(QK^T/sqrt(32)) @ V, then out_proj r->d_model.
  B=2, S=2048. mask is all-ones (spec fill), dropout p=0, biases zero but
  still applied.

Sharding (8 cores): core c handles batch b=c//4 and the 4 heads j=c%4
(128 of the 512 r-channels, column-parallel QKV). Attention is fully
local. A 4-way AllGather per q tile rebuilds attn_out (bf16), then each
core computes a 512-wide slice of d_model (column-parallel out_proj).

v2 structure (vs v1):
  - x and all weights are bf16 (host-cast): halves DMA and SBUF.
  - K,V projections run weight-stationary (dm-outer, 4 token tiles inner)
    so consecutive matmuls reuse the stationary operand; Q likewise.
  - Softmax denominator is folded into the AV matmul: the stationary is
    [V_chunk | ones] (64 cols), so out rows 0-31 are AV and 32-63 the
    denominator broadcast - the separate ones-matmul stream is gone.
  - exp is split between ACT (exact, bf16 out) and DVE (Schraudolph
    int16 bit-trick approximating exp in bf16), removing the Scalar
    engine as the per-tile bottleneck.
  - out_proj for tile q is emitted after attention of tile q+1 so the
    AllGather never head-of-line blocks the PE queue.
"""

import math

import numpy as np

B = 2
S = 2048
D_MODEL = 2048
R = 512
N_HEADS = 16
D_K = 32
N_CORES = 8
GROUP = 4          # cores per batch group
RLOC = 128         # r-channels per core (4 heads x 32)
NH = 4             # heads per core
TQ = 512           # q tile size
NQT = S // TQ      # 4 q tiles
NKT = S // 128     # 16 k chunks
NDM = D_MODEL // 128  # 16 d_model chunks

# Two-phase Schraudolph exp in bf16-bit space:
#   exp(s) ~= bitcast_bf16(int16(s*A + B1)) + bitcast_bf16(int16(s*A + B2))
# B1/B2 are half a bf16-exponent-period (64) apart and biased by -128 (a
# factor 0.5 each) so the SUM approximates exp(s); the -10.13 centers the
# ripple at zero mean (+-1.7% max, 0.56% rms vs 3% for single-phase).
EXPA = 184.66500854
EXPB1 = 16256.0 - 160.0 - 10.13
EXPB2 = 16256.0 - 96.0 - 10.13
AV_DELAY = 6      # AV consumption trails scores/exp by this many groups

_CACHE = {}
TRACE = False
LAST_RESULT = None


def _build():
    import concourse.mybir as mybir
    import concourse.tile as tile
    from concourse import bacc
    from concourse.masks import make_identity

    F32 = mybir.dt.float32
    F32R = mybir.dt.float32r
    BF16 = mybir.dt.bfloat16
    I16 = mybir.dt.int16

    nc = bacc.Bacc("TRN2", target_bir_lowering=False, num_devices=N_CORES)

    xT = nc.dram_tensor("xT", [D_MODEL, S], BF16, kind="ExternalInput")
    wqT = nc.dram_tensor("wqT", [D_MODEL, RLOC], BF16, kind="ExternalInput")
    wkT = nc.dram_tensor("wkT", [D_MODEL, RLOC], BF16, kind="ExternalInput")
    wvT = nc.dram_tensor("wvT", [D_MODEL, RLOC], BF16, kind="ExternalInput")
    woTs = nc.dram_tensor("woTs", [R, 512], BF16, kind="ExternalInput")
    bq = nc.dram_tensor("bq", [RLOC, 1], F32, kind="ExternalInput")
    bk = nc.dram_tensor("bk", [RLOC, 1], F32, kind="ExternalInput")
    bv = nc.dram_tensor("bv", [RLOC, 1], F32, kind="ExternalInput")
    bo2 = nc.dram_tensor("bo2", [128, 4], F32, kind="ExternalInput")
    outT = nc.dram_tensor("outT", [512, S], F32, kind="ExternalOutput")

    cc_in = [
        nc.dram_tensor(f"cc_in{i}", [RLOC, TQ], BF16, kind="Internal")
        for i in range(NQT)
    ]
    cc_out = [
        nc.dram_tensor(f"cc_out{i}", [R, TQ], BF16, kind="Internal")
        for i in range(NQT)
    ]
    cc_wi = nc.dram_tensor("cc_wi", [RLOC, TQ], BF16, kind="Internal")
    cc_wo = nc.dram_tensor("cc_wo", [R, TQ], BF16, kind="Internal")
    replica_groups = [[0, 1, 2, 3], [4, 5, 6, 7]]

    with tile.TileContext(nc) as tc:
        with (
            tc.tile_pool(name="const", bufs=1) as const,
            tc.tile_pool(name="wpool", bufs=1) as wpool,
            tc.tile_pool(name="xpool", bufs=1) as xpool,
            tc.tile_pool(name="qkv", bufs=1) as qkv,
            tc.tile_pool(name="attnp", bufs=8) as attnp,
            tc.tile_pool(name="expp", bufs=4) as expp,
            tc.tile_pool(name="denp", bufs=2) as denp,
            tc.tile_pool(name="otp", bufs=2) as otp,
            tc.tile_pool(name="agp", bufs=8) as agp,
            tc.tile_pool(name="outp", bufs=2) as outp,
        ):
            # ---- weight / bias / x loads ---------------------------------
            # interleave the DMA *issue* order (the issuing engine pays
            # ~0.6us per dma_start, so a long run of weight DMAs ahead of x
            # delays the first matmul by tens of us): x on the scalar
            # queue, wk/wv on sync, both dm-ascending.
            wk_sb = [
                wpool.tile([128, RLOC], BF16, name=f"wk_sb{dm}")
                for dm in range(NDM)
            ]
            wv_sb = [
                wpool.tile([128, RLOC], BF16, name=f"wv_sb{dm}")
                for dm in range(NDM)
            ]
            wq_sb = [
                wpool.tile([128, RLOC], BF16, name=f"wq_sb{dm}")
                for dm in range(NDM)
            ]
            x_sb = [
                xpool.tile([128, S], BF16, name=f"x_sb{dm}") for dm in range(NDM)
            ]
            for dm in range(NDM):
                rs = slice(128 * dm, 128 * (dm + 1))
                nc.scalar.dma_start(x_sb[dm][:], xT[rs, :])
                nc.sync.dma_start(wk_sb[dm][:], wkT[rs, :])
                nc.sync.dma_start(wv_sb[dm][:], wvT[rs, :])
            bq_sb = const.tile([RLOC, 1], F32)
            bk_sb = const.tile([RLOC, 1], F32)
            bv_sb = const.tile([RLOC, 1], F32)
            bo_sb = const.tile([128, 4], F32)
            nc.gpsimd.dma_start(bk_sb, bk[:])
            nc.gpsimd.dma_start(bv_sb, bv[:])
            nc.gpsimd.dma_start(bq_sb, bq[:])
            nc.gpsimd.dma_start(bo_sb, bo2[:])
            for dm in range(NDM):
                rs = slice(128 * dm, 128 * (dm + 1))
                nc.scalar.dma_start(wq_sb[dm][:], wqT[rs, :])
            wo_sb = wpool.tile([128, 4, 4, 128], BF16)
            for rc in range(4):
                for dmt in range(4):
                    nc.sync.dma_start(
                        wo_sb[:, rc, dmt, :],
                        woTs[128 * rc : 128 * (rc + 1), 128 * dmt : 128 * (dmt + 1)],
                    )

            ident = const.tile([128, 128], BF16)
            make_identity(nc, ident[:])
            nc.gpsimd.collective_compute(
                "AllGather",
                mybir.AluOpType.bypass,
                replica_groups=replica_groups,
                ins=[cc_wi[:]],
                outs=[cc_wo[:]],
            )

            kt = qkv.tile([RLOC, S], BF16)
            qt = qkv.tile([RLOC, S], BF16)
            vt_bf = qkv.tile([RLOC, S], BF16)
            # [V chunk | ones] stationaries: bf16 per-kc for the DVE-exp
            # groups, fp8 DoubleRow-interleaved per kc-pair for ACT groups
            v_aug = qkv.tile([128, NKT, NH, 64], BF16)
            nc.vector.memset(v_aug[:, :, :, 32:64], 1.0)

            # ---- single stable PSUM layout for the whole kernel ----------
            # sc: 3 x [128,1024] (6 banks) rotating: KV tiles, Q tiles,
            #     score groups, out_proj tiles all borrow this pool.
            # av: 2 x [128,512] (2 banks): V-transpose scratch, then the
            #     per-qtile AV+den accumulators.
            # Keeping one layout (no pool open/close between phases) keeps
            # the PE stream gap-free across phase transitions so the HAM
            # clock never re-throttles on a phase boundary.
            ps_sc_ctx = tc.tile_pool(name="ps_sc", bufs=3, space="PSUM")
            ps_av_ctx = tc.tile_pool(name="ps_av", bufs=2, space="PSUM")
            ps_sc = ps_sc_ctx.__enter__()
            ps_av = ps_av_ctx.__enter__()

            # K,V projections: token-tile-outer so each tile's K/V psum
            # drains (and V transposes run) while the next tile computes.
            for t in range(NQT):
                tsl = slice(TQ * t, TQ * (t + 1))
                kv = ps_sc.tile([128, 2 * TQ], F32, tag="sc", name=f"kv_ps{t}")
                for dm in range(NDM):
                    st = dm == 0
                    sp = dm == NDM - 1
                    nc.tensor.matmul(
                        kv[:, 0:TQ], wk_sb[dm][:],
                        x_sb[dm][:, tsl], start=st, stop=sp,
                    )
                    nc.tensor.matmul(
                        kv[:, TQ : 2 * TQ], wv_sb[dm][:],
                        x_sb[dm][:, tsl], start=st, stop=sp,
                    )
                nc.vector.tensor_scalar_add(kt[:, tsl], kv[:, 0:TQ], bk_sb[:])
                nc.vector.tensor_scalar_add(vt_bf[:, tsl], kv[:, TQ : 2 * TQ], bv_sb[:])
                # V^T -> [tok, chan] for this tile's 4 k-chunks
                tr = ps_av.tile([128, 4 * 128], BF16, tag="av", name=f"tr{t}")
                for c in range(4):
                    kc = 4 * t + c
                    nc.tensor.transpose(
                        tr[:, 128 * c : 128 * (c + 1)],
                        vt_bf[:, 128 * kc : 128 * (kc + 1)], ident[:],
                    )
                nc.vector.tensor_copy(
                    v_aug[:, 4 * t : 4 * (t + 1), :, 0:32],
                    tr[:].rearrange("p (c h d) -> p c h d", c=4, h=4),
                )

            # Q projection, token-tile-outer
            for t in range(NQT):
                tsl = slice(TQ * t, TQ * (t + 1))
                qp = ps_sc.tile([128, TQ], F32, tag="sc", name=f"q_ps{t}")
                for dm in range(NDM):
                    nc.tensor.matmul(
                        qp[:], wq_sb[dm][:], x_sb[dm][:, tsl],
                        start=(dm == 0), stop=(dm == NDM - 1),
                    )
                nc.vector.tensor_scalar_add(qt[:, tsl], qp[:], bq_sb[:])

            def out_proj(q):
                qsl = slice(TQ * q, TQ * (q + 1))
                ag_t = []
                for rc in range(GROUP):
                    t_ = agp.tile([128, TQ], BF16)
                    nc.sync.dma_start(t_, cc_out[q][128 * rc : 128 * (rc + 1), :])
                    ag_t.append(t_)
                for dmt in range(4):
                    pso2 = ps_sc.tile([128, TQ], F32, tag="sc", name=f"pso2_{q}_{dmt}")
                    for rc in range(GROUP):
                        nc.tensor.matmul(
                            pso2[:], wo_sb[:, rc, dmt, :], ag_t[rc][:],
                            start=(rc == 0), stop=(rc == GROUP - 1),
                        )
                    ob = outp.tile([128, TQ], F32)
                    nc.vector.tensor_scalar_add(
                        ob[:], pso2[:], bo_sb[:, dmt : dmt + 1]
                    )
                    nc.sync.dma_start(outT[128 * dmt : 128 * (dmt + 1), qsl], ob[:])

            for q in range(NQT):
                qsl = slice(TQ * q, TQ * (q + 1))
                av_ab = [
                    ps_av.tile([128, TQ], F32, tag="av", name=f"av_a{q}"),
                    ps_av.tile([128, TQ], F32, tag="av", name=f"av_b{q}"),
                ]
                # group g = (kc pair, head): scores for k chunks 2*kcp and
                # 2*kcp+1 of head h share one 2-bank psum tile. exp runs on
                # ACT (fp8 out, DoubleRow AV) for 3 of 4 groups and on DVE
                # (two-phase Schraudolph, bf16 AV) for the rest. The AV
                # stream trails by AV_DELAY groups so exp latency never
                # stalls the PE.
                NG = 2 * NKT          # 32 groups of [128, 1024]
                att_of = {}
                for gi in range(NG + AV_DELAY):
                    if gi < NG:
                        kc, p = divmod(gi, 2)
                        pss = ps_sc.tile([128, 2 * TQ], F32, tag="sc",
                                         name=f"pss{q}_{gi}")
                        for i in range(2):
                            h = 2 * p + i
                            nc.tensor.matmul(
                                pss[:, TQ * i : TQ * (i + 1)],
                                kt[32 * h : 32 * (h + 1), 128 * kc : 128 * (kc + 1)],
                                qt[32 * h : 32 * (h + 1), qsl],
                                start=True, stop=True,
                                tile_position=(32 * h, 0),
                            )
                        att = attnp.tile([128, 2 * TQ], BF16, tag="at",
                                         name=f"att_{q}_{gi}")
                        if (gi * 3) % 4 < 3:
                            nc.scalar.activation(
                                att[:], pss[:], mybir.ActivationFunctionType.Exp
                            )
                        else:
                            e1 = expp.tile([128, 2 * TQ], BF16)
                            e2 = expp.tile([128, 2 * TQ], BF16)
                            nc.vector.tensor_scalar(
                                e1[:].bitcast(I16), pss[:],
                                EXPA, EXPB1,
                                mybir.AluOpType.mult, mybir.AluOpType.add,
                            )
                            nc.vector.tensor_scalar(
                                e2[:].bitcast(I16), pss[:],
                                EXPA, EXPB2,
                                mybir.AluOpType.mult, mybir.AluOpType.add,
                            )
                            nc.vector.tensor_add(att[:], e1[:], e2[:])
                        att_of[gi] = att
                    ai = gi - AV_DELAY
                    if ai >= 0:
                        kc, p = divmod(ai, 2)
                        st = kc == 0
                        sp = kc == NKT - 1
                        att = att_of.pop(ai)
                        for i in range(2):
                            h = 2 * p + i
                            nc.tensor.matmul(
                                av_ab[p][64 * i : 64 * (i + 1), :],
                                v_aug[:, kc, h, :],
                                att[:, TQ * i : TQ * (i + 1)],
                                start=st, stop=sp,
                                tile_position=(0, 64 * i),
                            )

                # divide AV rows by the denominator rows (32 partitions up)
                ot = otp.tile([128, TQ], BF16)
                for half in range(2):
                    av = av_ab[half]
                    rb = denp.tile([128, TQ], F32)
                    nc.vector.reciprocal_approx_fast(rb[:], av[:])
                    for i in range(2):
                        nc.vector.tensor_mul(
                            ot[64 * half + 32 * i : 64 * half + 32 * (i + 1), :],
                            av[64 * i : 64 * i + 32, :],
                            rb[64 * i + 32 : 64 * (i + 1), :],
                        )
                nc.sync.dma_start(cc_in[q][:], ot[:])
                nc.gpsimd.collective_compute(
                    "AllGather",
                    mybir.AluOpType.bypass,
                    replica_groups=replica_groups,
                    ins=[cc_in[q][:]],
                    outs=[cc_out[q][:]],
                )
                if q >= 2:
                    out_proj(q - 2)
            out_proj(NQT - 2)
            out_proj(NQT - 1)

            ps_av_ctx.__exit__(None, None, None)
            ps_sc_ctx.__exit__(None, None, None)

    nc.finalize()
    return nc


def _prepare_inputs(x, Wq, bq, Wk, bk, Wv, bv, Wo, bo):
    import ml_dtypes

    bf16 = ml_dtypes.bfloat16
    scale = 1.0 / math.sqrt(D_K)
    x = np.asarray(x, np.float32)
    in_maps = []
    for c in range(N_CORES):
        b, j = divmod(c, GROUP)
        rsl = slice(RLOC * j, RLOC * (j + 1))
        dsl = slice(512 * j, 512 * (j + 1))
        in_maps.append(
            {
                "xT": np.ascontiguousarray(x[b].T).astype(bf16),
                "wqT": np.ascontiguousarray(
                    (np.asarray(Wq, np.float32)[rsl] * scale).T
                ).astype(bf16),
                "wkT": np.ascontiguousarray(
                    np.asarray(Wk, np.float32)[rsl].T
                ).astype(bf16),
                "wvT": np.ascontiguousarray(
                    np.asarray(Wv, np.float32)[rsl].T
                ).astype(bf16),
                "woTs": np.ascontiguousarray(
                    np.asarray(Wo, np.float32)[dsl].T
                ).astype(bf16),
                "bq": (np.asarray(bq)[rsl] * scale).astype(np.float32).reshape(RLOC, 1),
                "bk": np.asarray(bk)[rsl].astype(np.float32).reshape(RLOC, 1),
                "bv": np.asarray(bv)[rsl].astype(np.float32).reshape(RLOC, 1),
                "bo2": np.ascontiguousarray(
                    np.asarray(bo)[dsl].astype(np.float32).reshape(4, 128).T
                ),
            }
        )
    return in_maps


def kernel(x, Wq, bq, Wk, bk, Wv, bv, Wo, bo, mask=None):
    global LAST_RESULT
    from concourse.bass_utils import run_bass_kernel_spmd

    if "nc" not in _CACHE:
        _CACHE["nc"] = _build()
    nc = _CACHE["nc"]

    in_maps = _prepare_inputs(x, Wq, bq, Wk, bk, Wv, bv, Wo, bo)
    res = run_bass_kernel_spmd(
        nc, in_maps, core_ids=list(range(N_CORES)), trace=TRACE
    )
    LAST_RESULT = res
    out = np.empty((B, S, D_MODEL), np.float32)
    for c in range(N_CORES):
        b, j = divmod(c, GROUP)
        out[b, :, 512 * j : 512 * (j + 1)] = res.results[c]["outT"].T
    return out
